# revision 25
# baseline (speedup 1.0000x reference)
"""Additive (Bahdanau) attention on 8 trn2 NeuronCores.

Math (per batch element b, handled by one core):
  q_[tq, a]   = query[tq, :] @ Wq[a, :]          (A = 128 attention dim)
  k_[tk, a]   = key[tk, :]   @ Wk[a, :]
  logits[q,k] = sum_a w_w[a] * tanh(q_[q,a] + k_[k,a] + bias[a])   (+ w_b, which
                cancels in softmax and is therefore skipped)
  attn        = softmax_k(logits)
  out         = attn @ value

Mapping:
  - data-parallel over batch: core b <- batch b (B == n_cores == 8).
  - on-chip layout keeps A=128 on partitions for the tanh stage:
      k_pb  [a=128, TK=512]  (k projection + bias)
      q_pT  [a=128, TQ=512]
    per query q: DVE tensor_scalar_add broadcasts q_pT[:, q] over k_pb
    (fp32 2x mode), ScalarE evaluates tanh on [128, NB*512] blocks, and the
    PE reduces over A with a shifting zero-padded w_w window as the
    stationary operand so each query's logits land on psum partition q%128:
      matmul(lhsT = wbuf[:, 128-col:256-col], rhs = tanh_block)  ->
      psum[col, :] += w_w . tanh(...)
    accumulating logits^T(group) [128 q, 512 k] directly in one psum bank.
  - epilogue per 128-query group: Exp with accum_out rowsums (softmax without
    max-subtraction: logits ~ N(0, ~0.8), exp is exact to 2ulp on [-10, 10]),
    reciprocal + scale -> attn rows; PE transpose + matmul against value.

Sync-wait discipline: trn2 matmuls (which carry an implicit LDWEIGHTS) can
encode only ONE semaphore wait, so every instruction is arranged to depend on
at most one foreign engine beyond what its engine has already observed:
  - tiny PE "observer" matmuls at setup absorb each DMA/GPSIMD-produced
    tensor's semaphore one at a time;
  - wbuf is written only by DVE (memset + copy), never by DMA/GPSIMD;
  - a 1-element ACT absorber before each tanh carries the hid-slot
    WAR-on-PE wait;
  - per-group epilogue tiles get dedicated slots (bufs=4) so no slot is
    ever reused and no WAR-vs-DMA wait exists.
"""

import numpy as np

import concourse.bass as bass
import concourse.tile as tile
from concourse import bacc, mybir
from concourse.bass_utils import run_bass_kernel_spmd

F32 = mybir.dt.float32
AF = mybir.ActivationFunctionType

B, TQ, TK, DQ, DK, DV, A = 8, 512, 512, 512, 512, 512, 128
NB = 8           # queries per DVE/ScalarE block
N_SUMS_BUFS = 2
N_HID_BUFS = 2
N_CORES = 8

_CACHE = {}

_add_dep_helper = bass._add_dep_helper
_DEP_SYNC = mybir.DependencyInfo.SYNC_ONLY
_DEP_NOSYNC = mybir.DependencyInfo.NO_SYNC_ONLY


def _demote(ins, dep_ins):
    """Demote a sync dependency edge to a no-sync (ordering-only) edge."""
    if ins.try_remove_dependency(dep_ins.name, _DEP_SYNC):
        ins.add_dependency(dep_ins.name, _DEP_NOSYNC)


def build_nc():
    nc = bacc.Bacc(None, target_bir_lowering=False, debug=False)

    qT = nc.declare_dram_parameter("qT", [DQ, TQ], F32, isOutput=False)
    kT = nc.declare_dram_parameter("kT", [DK, TK], F32, isOutput=False)
    val = nc.declare_dram_parameter("value", [TK, DV], F32, isOutput=False)
    WqT = nc.declare_dram_parameter("WqT", [DQ, A], F32, isOutput=False)
    WkT = nc.declare_dram_parameter("WkT", [DK, A], F32, isOutput=False)
    bias = nc.declare_dram_parameter("bias", [A, 1], F32, isOutput=False)
    # host-precomputed: zeros with w_w in column 128 (matvec weight window)
    wbuf_d = nc.declare_dram_parameter("wbuf", [128, 256], F32, isOutput=False)
    # host-precomputed 128x128 identity (PE transpose operand)
    ident_d = nc.declare_dram_parameter("ident", [128, 128], F32, isOutput=False)
    attn_out = nc.declare_dram_parameter("attn", [TQ, TK], F32, isOutput=True)
    out_out = nc.declare_dram_parameter("out", [TQ, DV], F32, isOutput=True)

    n_groups = TQ // 128
    blocks_per_group = 128 // NB

    with tile.TileContext(nc) as tc:
        with (
            tc.tile_pool(name="pers", bufs=1) as pers,
            tc.tile_pool(name="stage", bufs=1) as stage,
            tc.tile_pool(name="sums", bufs=2) as sums_pool,
            tc.tile_pool(name="hid", bufs=2) as hid_pool,
            tc.tile_pool(name="epi", bufs=4) as epi_pool,
            tc.tile_pool(name="small", bufs=4) as small_pool,
            tc.tile_pool(name="attnT", bufs=2) as attnT_pool,
            tc.tile_pool(name="plog", bufs=2, space="PSUM") as plog_pool,
            tc.tile_pool(name="pout", bufs=2, space="PSUM") as pout_pool,
            tc.tile_pool(name="ptr", bufs=2, space="PSUM") as ptr_pool,
            tc.tile_pool(name="pproj", bufs=1, space="PSUM") as pproj_pool,
            tc.tile_pool(name="pobs", bufs=1, space="PSUM") as pobs_pool,
        ):
            # ---- persistent tiles ----
            value_sb = pers.tile([128, TK // 128, DV], F32)
            q_pT = pers.tile([128, TQ], F32)
            k_pb = pers.tile([128, TK], F32)
            wbuf = pers.tile([128, 256], F32)
            bias_sb = pers.tile([128, 1], F32)
            ident = pers.tile([128, 128], F32)
            scr11 = pers.tile([1, 1], F32)
            scr11b = pers.tile([1, 1], F32)

            nc.sync.dma_start(out=wbuf[:], in_=wbuf_d[:, :])
            nc.sync.dma_start(out=ident[:], in_=ident_d[:, :])
            nc.sync.dma_start(out=bias_sb[:, :], in_=bias[:, :])
            nc.sync.dma_start(
                out=value_sb[:], in_=val.rearrange("(c p) d -> p c d", p=128)
            )

            # ---- staged inputs for the projections ----
            WqT_sb = stage.tile([128, DQ // 128, A], F32)
            WkT_sb = stage.tile([128, DK // 128, A], F32)
            qT_sb = stage.tile([128, DQ // 128, TQ], F32)
            kT_sb = stage.tile([128, DK // 128, TK], F32)
            nc.sync.dma_start(out=WqT_sb[:], in_=WqT.rearrange("(c p) a -> p c a", p=128))
            nc.sync.dma_start(out=WkT_sb[:], in_=WkT.rearrange("(c p) a -> p c a", p=128))
            nc.sync.dma_start(out=qT_sb[:], in_=qT.rearrange("(c p) t -> p c t", p=128))
            nc.sync.dma_start(out=kT_sb[:], in_=kT.rearrange("(c p) t -> p c t", p=128))

            # ---- PE observers: absorb one foreign semaphore each so that no
            # later matmul needs more than one sync wait ----
            obs = pobs_pool.tile([1, 1], F32)
            for src in (
                WqT_sb[:, 0, 0:1],
                WkT_sb[:, 0, 0:1],
                qT_sb[:, 0, 0:1],
                kT_sb[:, 0, 0:1],
                value_sb[:, 0, 0:1],
                ident[:, 0:1],
                wbuf[:, 0:1],
            ):
                nc.tensor.matmul(obs[:], src, src, start=True, stop=True)
            # ACT observers for the bias/wbuf DMAs (later ACT instructions that
            # reference them must only wait on their data producer).
            nc.scalar.copy(scr11[:], bias_sb[0:1, 0:1])
            nc.scalar.copy(scr11b[:], wbuf[0:1, 0:1])
            # all-zeros column used as an explicit AP bias for Tanh/Exp so bass
            # does not materialize a const-AP (which would add a second wait)
            zbias = wbuf[:, 0:1]

            # ---- projections: q_pT[a, tq], k_pb[a, tk] ----
            q_ps = pproj_pool.tile([128, TQ], F32, tag="proj")
            for c in range(DQ // 128):
                nc.tensor.matmul(
                    q_ps[:], WqT_sb[:, c, :], qT_sb[:, c, :],
                    start=(c == 0), stop=(c == DQ // 128 - 1),
                )
            nc.scalar.copy(q_pT[:], q_ps[:])

            k_ps = pproj_pool.tile([128, TK], F32, tag="proj")
            for c in range(DK // 128):
                nc.tensor.matmul(
                    k_ps[:], WkT_sb[:, c, :], kT_sb[:, c, :],
                    start=(c == 0), stop=(c == DK // 128 - 1),
                )
            # fold the additive bias into the psum -> sbuf copy
            nc.scalar.activation(
                k_pb[:], k_ps[:], AF.Identity, bias=bias_sb[:, 0:1], scale=1.0
            )

            # ---- main loop ----
            # Per block of NB queries: 16/NB DVE tensor_scalar_adds (fp32 2x
            # mode) broadcast query columns over k_pb, one big ScalarE tanh,
            # then NB matvec matmuls accumulate logits^T into psum.
            #
            # The trn2 ISA encodes at most ONE semaphore wait per instruction
            # and Tile does not exploit transitive observation, so the slot
            # recycling hazards are routed by hand:
            #   - add[1] of each block carries an explicit sync dep on the
            #     last matmul that read the hid slot being recycled;
            #   - the tanh's own WAR/WAW deps on that old block (implied
            #     transitively through add[1]'s fence) are demoted to
            #     no-sync edges (ordering only, no semaphore);
            #   - ditto for the DVE adds' WAW on the recycled sums slot
            #     (implied through the WAR on the old tanh's read).
            add_hist = {}   # bidx -> list of add instructions
            tanh_hist = {}  # bidx -> tanh instruction
            mm_hist = {}    # bidx -> list of matmul instructions
            n_blocks_total = n_groups * blocks_per_group
            for g in range(n_groups):
                plog = plog_pool.tile([128, TK], F32)
                for blk in range(blocks_per_group):
                    bidx = g * blocks_per_group + blk
                    sums = sums_pool.tile([128, NB * TK], F32)
                    adds = []
                    for jj in range(NB):
                        q = g * 128 + blk * NB + jj
                        h = nc.vector.tensor_scalar_add(
                            sums[:, jj * TK:(jj + 1) * TK],
                            k_pb[:], q_pT[:, q:q + 1],
                        )
                        adds.append(h.ins)
                        # WAW vs the add that wrote this sums slot 2 blocks
                        # ago: implied through the WAR on the old tanh's read.
                        if bidx >= N_SUMS_BUFS:
                            for old in add_hist[bidx - N_SUMS_BUFS]:
                                _demote(h.ins, old)
                    add_hist[bidx] = adds
                    if bidx >= N_HID_BUFS:
                        # fence: carry the hid-slot recycle wait (PE) on an
                        # otherwise wait-free add
                        _add_dep_helper(
                            adds[1], mm_hist[bidx - N_HID_BUFS][-1], sync=True,
                            reason="hid slot recycle fence",
                        )
                    hid = hid_pool.tile([128, NB * TK], F32)
                    th = nc.scalar.activation(hid[:], sums[:], AF.Tanh, bias=zbias)
                    if bidx >= N_HID_BUFS:
                        old = bidx - N_HID_BUFS
                        _demote(th.ins, tanh_hist[old])
                        for mi in mm_hist[old]:
                            _demote(th.ins, mi)
                    tanh_hist[bidx] = th.ins
                    mms = []
                    for jj in range(NB):
                        col = blk * NB + jj
                        m = nc.tensor.matmul(
                            plog[:],
                            wbuf[:, 128 - col:256 - col],
                            hid[:, jj * TK:(jj + 1) * TK],
                            start=(col == 0), stop=(col == 127),
                        )
                        mms.append(m.ins)
                    mm_hist[bidx] = mms

                # ---- epilogue for this 128-query group ----
                expt = epi_pool.tile([128, TK], F32)
                rowsum = small_pool.tile([128, 1], F32)
                nc.scalar.activation(
                    expt[:], plog[:], AF.Exp, bias=zbias, accum_out=rowsum[:, :]
                )
                recip = small_pool.tile([128, 1], F32)
                nc.vector.reciprocal(recip[:, :], rowsum[:, :])
                attn_sb = epi_pool.tile([128, TK], F32)
                nc.vector.tensor_scalar_mul(attn_sb[:], expt[:], recip[:, 0:1])
                nc.sync.dma_start(
                    out=attn_out[g * 128:(g + 1) * 128, :], in_=attn_sb[:]
                )

                out_ps = pout_pool.tile([128, DV], F32)
                for c in range(TK // 128):
                    tp = ptr_pool.tile([128, 128], F32)
                    nc.tensor.transpose(
                        tp[:], attn_sb[:, c * 128:(c + 1) * 128], ident[:]
                    )
                    attnT = attnT_pool.tile([128, 128], F32)
                    nc.vector.tensor_copy(attnT[:], tp[:])
                    nc.tensor.matmul(
                        out_ps[:], attnT[:], value_sb[:, c, :],
                        start=(c == 0), stop=(c == TK // 128 - 1),
                    )
                out_sb = epi_pool.tile([128, DV], F32)
                nc.vector.tensor_copy(out_sb[:], out_ps[:])
                nc.sync.dma_start(
                    out=out_out[g * 128:(g + 1) * 128, :], in_=out_sb[:]
                )

    nc.compile()
    return nc


def _get_nc():
    if "nc" not in _CACHE:
        _CACHE["nc"] = build_nc()
    return _CACHE["nc"]


def make_in_maps(query, key, value, Wq, Wk, bias, w_w, **_):
    WqT = np.ascontiguousarray(Wq.T, dtype=np.float32)
    WkT = np.ascontiguousarray(Wk.T, dtype=np.float32)
    bias_c = np.ascontiguousarray(bias.reshape(A, 1), dtype=np.float32)
    wbuf_np = np.zeros((128, 256), dtype=np.float32)
    wbuf_np[:, 128] = np.asarray(w_w, dtype=np.float32).reshape(A)
    ident_np = np.eye(128, dtype=np.float32)
    in_maps = []
    for b in range(B):
        in_maps.append({
            "qT": np.ascontiguousarray(query[b].T, dtype=np.float32),
            "kT": np.ascontiguousarray(key[b].T, dtype=np.float32),
            "value": np.ascontiguousarray(value[b], dtype=np.float32),
            "WqT": WqT,
            "WkT": WkT,
            "bias": bias_c,
            "wbuf": wbuf_np,
            "ident": ident_np,
        })
    return in_maps


def run(inputs, trace=False, **kwargs):
    nc = _get_nc()
    in_maps = make_in_maps(**{k: np.asarray(v) for k, v in inputs.items()})
    res = run_bass_kernel_spmd(
        nc, in_maps, list(range(N_CORES)), trace=trace, **kwargs
    )
    output = np.stack([res.results[b]["out"] for b in range(B)])
    attn = np.stack([res.results[b]["attn"] for b in range(B)])
    return (output, attn), res


def kernel(**inputs):
    (output, attn), _ = run(inputs)
    return output, attn


# revision 33
# speedup vs baseline: 1.4693x; 1.4693x over previous
"""Additive (Bahdanau) attention on 8 trn2 NeuronCores.

Math (per batch element b, handled by one core):
  q_[tq, a]   = query[tq, :] @ Wq[a, :]          (A = 128 attention dim)
  k_[tk, a]   = key[tk, :]   @ Wk[a, :]
  logits[q,k] = sum_a w_w[a] * tanh(q_[q,a] + k_[k,a] + bias[a])   (+ w_b, which
                cancels in softmax and is therefore skipped)
  attn        = softmax_k(logits)
  out         = attn @ value

Mapping:
  - data-parallel over batch: core b <- batch b (B == n_cores == 8).
  - on-chip layout keeps A=128 on partitions for the tanh stage:
      k_pb  [a=128, TK=512]  (k projection + bias)
      q_pT  [a=128, TQ=512]
    per query q: DVE tensor_scalar_add broadcasts q_pT[:, q] over k_pb
    (fp32 2x mode), ScalarE evaluates tanh on [128, NB*512] blocks, and the
    PE reduces over A with a shifting zero-padded w_w window as the
    stationary operand so each query's logits land on psum partition q%128:
      matmul(lhsT = wbuf[:, 128-col:256-col], rhs = tanh_block)  ->
      psum[col, :] += w_w . tanh(...)
    accumulating logits^T(group) [128 q, 512 k] directly in one psum bank.
  - epilogue per 128-query group: Exp with accum_out rowsums (softmax without
    max-subtraction: logits ~ N(0, ~0.8), exp is exact to 2ulp on [-10, 10]),
    reciprocal + scale -> attn rows; PE transpose + matmul against value.

Sync-wait discipline: trn2 matmuls (which carry an implicit LDWEIGHTS) can
encode only ONE semaphore wait, so every instruction is arranged to depend on
at most one foreign engine beyond what its engine has already observed:
  - tiny PE "observer" matmuls at setup absorb each DMA/GPSIMD-produced
    tensor's semaphore one at a time;
  - wbuf is written only by DVE (memset + copy), never by DMA/GPSIMD;
  - a 1-element ACT absorber before each tanh carries the hid-slot
    WAR-on-PE wait;
  - per-group epilogue tiles get dedicated slots (bufs=4) so no slot is
    ever reused and no WAR-vs-DMA wait exists.
"""

import numpy as np

import concourse.bass as bass
import concourse.tile as tile
from concourse import bacc, mybir
from concourse.bass_utils import run_bass_kernel_spmd

F32 = mybir.dt.float32
F32R = mybir.dt.float32r
AF = mybir.ActivationFunctionType

B, TQ, TK, DQ, DK, DV, A = 8, 512, 512, 512, 512, 512, 128
NB = 8           # queries per DVE/ScalarE block
N_SUMS_BUFS = 2
N_HID_BUFS = 2
N_CORES = 8

_CACHE = {}

_add_dep_helper = bass._add_dep_helper
_DEP_SYNC = mybir.DependencyInfo.SYNC_ONLY
_DEP_NOSYNC = mybir.DependencyInfo.NO_SYNC_ONLY


def _demote(ins, dep_ins):
    """Demote a sync dependency edge to a no-sync (ordering-only) edge."""
    if ins.try_remove_dependency(dep_ins.name, _DEP_SYNC):
        ins.add_dependency(dep_ins.name, _DEP_NOSYNC)


def build_nc():
    nc = bacc.Bacc(None, target_bir_lowering=False, debug=False)

    qT = nc.declare_dram_parameter("qT", [DQ, TQ], F32, isOutput=False)
    kT = nc.declare_dram_parameter("kT", [DK, TK], F32, isOutput=False)
    val = nc.declare_dram_parameter("value", [TK, DV], F32, isOutput=False)
    WqT = nc.declare_dram_parameter("WqT", [DQ, A], F32, isOutput=False)
    WkT = nc.declare_dram_parameter("WkT", [DK, A], F32, isOutput=False)
    bias = nc.declare_dram_parameter("bias", [A, 1], F32, isOutput=False)
    # host-precomputed: zeros with w_w in column 128 (matvec weight window)
    wbuf_d = nc.declare_dram_parameter("wbuf", [128, 256], F32R, isOutput=False)
    # host-precomputed 128x128 identity (PE transpose operand)
    ident_d = nc.declare_dram_parameter("ident", [128, 128], F32, isOutput=False)
    attn_out = nc.declare_dram_parameter("attn", [TQ, TK], F32, isOutput=True)
    out_out = nc.declare_dram_parameter("out", [TQ, DV], F32, isOutput=True)

    n_groups = TQ // 128
    blocks_per_group = 128 // NB

    with tile.TileContext(nc) as tc:
        with (
            tc.tile_pool(name="pers", bufs=1) as pers,
            tc.tile_pool(name="stage", bufs=1) as stage,
            tc.tile_pool(name="sums", bufs=2) as sums_pool,
            tc.tile_pool(name="hid", bufs=2) as hid_pool,
            tc.tile_pool(name="epi", bufs=4) as epi_pool,
            tc.tile_pool(name="small", bufs=4) as small_pool,
            tc.tile_pool(name="attnT", bufs=2) as attnT_pool,
            tc.tile_pool(name="plog", bufs=2, space="PSUM") as plog_pool,
            tc.tile_pool(name="pout", bufs=2, space="PSUM") as pout_pool,
            tc.tile_pool(name="ptr", bufs=2, space="PSUM") as ptr_pool,
            tc.tile_pool(name="pproj", bufs=1, space="PSUM") as pproj_pool,
            tc.tile_pool(name="pobs", bufs=1, space="PSUM") as pobs_pool,
        ):
            # ---- persistent tiles ----
            value_sb = pers.tile([128, TK // 128, DV], F32)
            q_pT = pers.tile([128, TQ], F32)
            k_pb = pers.tile([128, TK], F32)
            wbuf = pers.tile([128, 256], F32R)
            bias_sb = pers.tile([128, 1], F32)
            ident = pers.tile([128, 128], F32)
            scr11 = pers.tile([1, 1], F32)
            scr11b = pers.tile([1, 1], F32)

            nc.sync.dma_start(out=wbuf[:], in_=wbuf_d[:, :])
            nc.sync.dma_start(out=ident[:], in_=ident_d[:, :])
            nc.sync.dma_start(out=bias_sb[:, :], in_=bias[:, :])
            nc.sync.dma_start(
                out=value_sb[:], in_=val.rearrange("(c p) d -> p c d", p=128)
            )

            # ---- staged inputs for the projections ----
            WqT_sb = stage.tile([128, DQ // 128, A], F32)
            WkT_sb = stage.tile([128, DK // 128, A], F32)
            qT_sb = stage.tile([128, DQ // 128, TQ], F32)
            kT_sb = stage.tile([128, DK // 128, TK], F32)
            nc.sync.dma_start(out=WqT_sb[:], in_=WqT.rearrange("(c p) a -> p c a", p=128))
            nc.sync.dma_start(out=WkT_sb[:], in_=WkT.rearrange("(c p) a -> p c a", p=128))
            nc.sync.dma_start(out=qT_sb[:], in_=qT.rearrange("(c p) t -> p c t", p=128))
            nc.sync.dma_start(out=kT_sb[:], in_=kT.rearrange("(c p) t -> p c t", p=128))

            # ---- PE observers: absorb one foreign semaphore each so that no
            # later matmul needs more than one sync wait ----
            obs = pobs_pool.tile([1, 1], F32)
            for src in (
                WqT_sb[:, 0, 0:1],
                WkT_sb[:, 0, 0:1],
                qT_sb[:, 0, 0:1],
                kT_sb[:, 0, 0:1],
                value_sb[:, 0, 0:1],
                ident[:, 0:1],
                wbuf[:, 0:1].bitcast(F32),
            ):
                nc.tensor.matmul(obs[:], src, src, start=True, stop=True)
            # ACT observers for the bias/wbuf DMAs (later ACT instructions that
            # reference them must only wait on their data producer).
            nc.scalar.copy(scr11[:], bias_sb[0:1, 0:1])
            nc.scalar.copy(scr11b[:], wbuf[0:1, 0:1])
            # all-zeros column used as an explicit AP bias for Tanh/Exp so bass
            # does not materialize a const-AP (which would add a second wait)
            zbias = wbuf[:, 0:1].bitcast(F32)

            # ---- projections: q_pT[a, tq], k_pb[a, tk] ----
            q_ps = pproj_pool.tile([128, TQ], F32, tag="proj")
            for c in range(DQ // 128):
                nc.tensor.matmul(
                    q_ps[:], WqT_sb[:, c, :], qT_sb[:, c, :],
                    start=(c == 0), stop=(c == DQ // 128 - 1),
                )
            nc.scalar.copy(q_pT[:], q_ps[:])

            k_ps = pproj_pool.tile([128, TK], F32, tag="proj")
            for c in range(DK // 128):
                nc.tensor.matmul(
                    k_ps[:], WkT_sb[:, c, :], kT_sb[:, c, :],
                    start=(c == 0), stop=(c == DK // 128 - 1),
                )
            # fold the additive bias into the psum -> sbuf copy
            nc.scalar.activation(
                k_pb[:], k_ps[:], AF.Identity, bias=bias_sb[:, 0:1], scale=1.0
            )

            # ---- main loop ----
            # Per block of NB queries: 16/NB DVE tensor_scalar_adds (fp32 2x
            # mode) broadcast query columns over k_pb, one big ScalarE tanh,
            # then NB matvec matmuls accumulate logits^T into psum.
            #
            # The trn2 ISA encodes at most ONE semaphore wait per instruction
            # and Tile does not exploit transitive observation, so the slot
            # recycling hazards are routed by hand:
            #   - add[1] of each block carries an explicit sync dep on the
            #     last matmul that read the hid slot being recycled;
            #   - the tanh's own WAR/WAW deps on that old block (implied
            #     transitively through add[1]'s fence) are demoted to
            #     no-sync edges (ordering only, no semaphore);
            #   - ditto for the DVE adds' WAW on the recycled sums slot
            #     (implied through the WAR on the old tanh's read).
            add_hist = {}   # bidx -> list of add instructions
            tanh_hist = {}  # bidx -> tanh instruction
            mm_hist = {}    # bidx -> list of matmul instructions
            n_blocks_total = n_groups * blocks_per_group
            for g in range(n_groups):
                plog = plog_pool.tile([128, TK], F32)
                for blk in range(blocks_per_group):
                    bidx = g * blocks_per_group + blk
                    sums = sums_pool.tile([128, NB * TK], F32)
                    adds = []
                    for jj in range(NB):
                        q = g * 128 + blk * NB + jj
                        h = nc.vector.tensor_scalar_add(
                            sums[:, jj * TK:(jj + 1) * TK],
                            k_pb[:], q_pT[:, q:q + 1],
                        )
                        adds.append(h.ins)
                        # WAW vs the add that wrote this sums slot 2 blocks
                        # ago: implied through the WAR on the old tanh's read.
                        if bidx >= N_SUMS_BUFS:
                            for old in add_hist[bidx - N_SUMS_BUFS]:
                                _demote(h.ins, old)
                    add_hist[bidx] = adds
                    if bidx >= N_HID_BUFS:
                        # fence: carry the hid-slot recycle wait (PE) on an
                        # otherwise wait-free add
                        _add_dep_helper(
                            adds[1], mm_hist[bidx - N_HID_BUFS][-1], sync=True,
                            reason="hid slot recycle fence",
                        )
                    hid = hid_pool.tile([128, NB * TK], F32R)
                    th = nc.scalar.activation(hid[:], sums[:], AF.Tanh, bias=zbias)
                    if bidx >= N_HID_BUFS:
                        old = bidx - N_HID_BUFS
                        _demote(th.ins, tanh_hist[old])
                        for mi in mm_hist[old]:
                            _demote(th.ins, mi)
                    tanh_hist[bidx] = th.ins
                    mms = []
                    for jj in range(NB):
                        col = blk * NB + jj
                        # float32r = single-pass reduced-precision fp32 on the
                        # PE (plain fp32 streams every column twice)
                        m = nc.tensor.matmul(
                            plog[:],
                            wbuf[:, 128 - col:256 - col],
                            hid[:, jj * TK:(jj + 1) * TK],
                            start=(col == 0), stop=(col == 127),
                        )
                        mms.append(m.ins)
                    mm_hist[bidx] = mms

                # ---- epilogue for this 128-query group ----
                expt = epi_pool.tile([128, TK], F32)
                rowsum = small_pool.tile([128, 1], F32)
                nc.scalar.activation(
                    expt[:], plog[:], AF.Exp, bias=zbias, accum_out=rowsum[:, :]
                )
                recip = small_pool.tile([128, 1], F32)
                nc.vector.reciprocal(recip[:, :], rowsum[:, :])
                attn_sb = epi_pool.tile([128, TK], F32)
                nc.vector.tensor_scalar_mul(attn_sb[:], expt[:], recip[:, 0:1])
                nc.sync.dma_start(
                    out=attn_out[g * 128:(g + 1) * 128, :], in_=attn_sb[:]
                )

                out_ps = pout_pool.tile([128, DV], F32)
                for c in range(TK // 128):
                    tp = ptr_pool.tile([128, 128], F32)
                    nc.tensor.transpose(
                        tp[:], attn_sb[:, c * 128:(c + 1) * 128], ident[:]
                    )
                    attnT = attnT_pool.tile([128, 128], F32)
                    nc.vector.tensor_copy(attnT[:], tp[:])
                    nc.tensor.matmul(
                        out_ps[:], attnT[:], value_sb[:, c, :],
                        start=(c == 0), stop=(c == TK // 128 - 1),
                    )
                out_sb = epi_pool.tile([128, DV], F32)
                nc.vector.tensor_copy(out_sb[:], out_ps[:])
                nc.sync.dma_start(
                    out=out_out[g * 128:(g + 1) * 128, :], in_=out_sb[:]
                )

    nc.compile()
    return nc


def _get_nc():
    if "nc" not in _CACHE:
        _CACHE["nc"] = build_nc()
    return _CACHE["nc"]


def make_in_maps(query, key, value, Wq, Wk, bias, w_w, **_):
    WqT = np.ascontiguousarray(Wq.T, dtype=np.float32)
    WkT = np.ascontiguousarray(Wk.T, dtype=np.float32)
    bias_c = np.ascontiguousarray(bias.reshape(A, 1), dtype=np.float32)
    wbuf_np = np.zeros((128, 256), dtype=np.float32)
    wbuf_np[:, 128] = np.asarray(w_w, dtype=np.float32).reshape(A)
    ident_np = np.eye(128, dtype=np.float32)
    in_maps = []
    for b in range(B):
        in_maps.append({
            "qT": np.ascontiguousarray(query[b].T, dtype=np.float32),
            "kT": np.ascontiguousarray(key[b].T, dtype=np.float32),
            "value": np.ascontiguousarray(value[b], dtype=np.float32),
            "WqT": WqT,
            "WkT": WkT,
            "bias": bias_c,
            "wbuf": wbuf_np,
            "ident": ident_np,
        })
    return in_maps


def run(inputs, trace=False, **kwargs):
    nc = _get_nc()
    in_maps = make_in_maps(**{k: np.asarray(v) for k, v in inputs.items()})
    res = run_bass_kernel_spmd(
        nc, in_maps, list(range(N_CORES)), trace=trace, **kwargs
    )
    output = np.stack([res.results[b]["out"] for b in range(B)])
    attn = np.stack([res.results[b]["attn"] for b in range(B)])
    return (output, attn), res


def kernel(**inputs):
    (output, attn), _ = run(inputs)
    return output, attn


# revision 34
# speedup vs baseline: 1.6739x; 1.1392x over previous
"""Additive (Bahdanau) attention on 8 trn2 NeuronCores.

Math (per batch element b, handled by one core):
  q_[tq, a]   = query[tq, :] @ Wq[a, :]          (A = 128 attention dim)
  k_[tk, a]   = key[tk, :]   @ Wk[a, :]
  logits[q,k] = sum_a w_w[a] * tanh(q_[q,a] + k_[k,a] + bias[a])   (+ w_b, which
                cancels in softmax and is therefore skipped)
  attn        = softmax_k(logits)
  out         = attn @ value

Mapping (one batch element per core, 8 cores):
  - A=128 lives on partitions for the tanh stage:
      k_pb  [a=128, TK=512]  (k projection + bias),  q_pT [a=128, TQ=512]
  - per query q the tanh argument k_pb + q_pT[:, q] is built either by a DVE
    tensor_scalar_add into a block buffer followed by one big ScalarE Tanh
    (batched blocks), or fused directly into the ScalarE activation as a
    per-partition bias (fused blocks). The split is chosen to balance DVE
    and ScalarE busy time.
  - tanh output is bf16; the PE reduces over A with a shifting zero-padded
    bf16 w_w window as the stationary operand (FWL-fast weight loads):
      matmul(lhsT = wbuf[:, 128-col:256-col], rhs = tanh_block)
    accumulating logits^T(group) [128 q, 512 k] into one psum bank, query
    col = psum partition.
  - epilogue per 128-query group: Exp with accum_out rowsums (softmax without
    max-subtraction: logits ~ N(0, ~0.8), exp exact to 2ulp on [-10, 10]),
    DVE reciprocal + scale -> attn rows; PE transpose + matmul against value.

Sync-wait discipline: every trn2 instruction encodes at most ONE semaphore
wait (bacc's event-semaphore pass can legalize more, but each extra wait
costs time), and Tile does not track transitive observation. So:
  - tiny PE observer matmuls at setup absorb each DMA-produced tensor's
    semaphore one at a time;
  - slot-recycling hazards on the sums/hid pools are carried by a single
    designated instruction per block and the transitively-implied duplicate
    edges are demoted to no-sync (ordering-only) edges.
"""

import numpy as np
import ml_dtypes

import concourse.bass as bass
import concourse.tile as tile
from concourse import bacc, mybir
from concourse.bass_utils import run_bass_kernel_spmd

F32 = mybir.dt.float32
BF16 = mybir.dt.bfloat16
AF = mybir.ActivationFunctionType

B, TQ, TK, DQ, DK, DV, A = 8, 512, 512, 512, 512, 512, 128
NB = 16          # queries per block
N_SUMS_BUFS = 2
N_HID_BUFS = 2
# number of fused-ACT blocks per 128-query group (rest are DVE+batched-tanh);
# chosen to balance ScalarE vs VectorE busy time
FUSED_BLOCKS = [1, 2, 1, 2]
N_CORES = 8

_CACHE = {}

_add_dep_helper = bass._add_dep_helper
_DEP_SYNC = mybir.DependencyInfo.SYNC_ONLY
_DEP_NOSYNC = mybir.DependencyInfo.NO_SYNC_ONLY


def _demote(ins, dep_ins):
    """Demote a sync dependency edge to a no-sync (ordering-only) edge."""
    if ins.try_remove_dependency(dep_ins.name, _DEP_SYNC):
        ins.add_dependency(dep_ins.name, _DEP_NOSYNC)


def build_nc():
    nc = bacc.Bacc(None, target_bir_lowering=False, debug=False)

    qT = nc.declare_dram_parameter("qT", [DQ, TQ], F32, isOutput=False)
    kT = nc.declare_dram_parameter("kT", [DK, TK], F32, isOutput=False)
    val = nc.declare_dram_parameter("value", [TK, DV], F32, isOutput=False)
    WqT = nc.declare_dram_parameter("WqT", [DQ, A], F32, isOutput=False)
    WkT = nc.declare_dram_parameter("WkT", [DK, A], F32, isOutput=False)
    # column 0: additive bias; column 1: zeros (used as AP bias for Tanh/Exp
    # so bass does not materialize a const-AP, which would cost a 2nd wait)
    bias = nc.declare_dram_parameter("bias", [A, 2], F32, isOutput=False)
    # host-precomputed: zeros with w_w in column 128 (matvec weight window)
    wbuf_d = nc.declare_dram_parameter("wbuf", [128, 256], BF16, isOutput=False)
    # host-precomputed 128x128 identity (PE transpose operand)
    ident_d = nc.declare_dram_parameter("ident", [128, 128], F32, isOutput=False)
    attn_out = nc.declare_dram_parameter("attn", [TQ, TK], F32, isOutput=True)
    out_out = nc.declare_dram_parameter("out", [TQ, DV], F32, isOutput=True)

    n_groups = TQ // 128
    blocks_per_group = 128 // NB

    with tile.TileContext(nc) as tc:
        with (
            tc.tile_pool(name="pers", bufs=1) as pers,
            tc.tile_pool(name="stage", bufs=1) as stage,
            tc.tile_pool(name="sums", bufs=N_SUMS_BUFS) as sums_pool,
            tc.tile_pool(name="hid", bufs=N_HID_BUFS) as hid_pool,
            tc.tile_pool(name="epi", bufs=4) as epi_pool,
            tc.tile_pool(name="small", bufs=4) as small_pool,
            tc.tile_pool(name="attnT", bufs=2) as attnT_pool,
            tc.tile_pool(name="plog", bufs=2, space="PSUM") as plog_pool,
            tc.tile_pool(name="pout", bufs=2, space="PSUM") as pout_pool,
            tc.tile_pool(name="ptr", bufs=2, space="PSUM") as ptr_pool,
            tc.tile_pool(name="pproj", bufs=1, space="PSUM") as pproj_pool,
            tc.tile_pool(name="pobs", bufs=1, space="PSUM") as pobs_pool,
        ):
            # ---- persistent tiles ----
            value_sb = pers.tile([128, TK // 128, DV], F32)
            q_pT = pers.tile([128, TQ], F32)
            k_pb = pers.tile([128, TK], F32)
            wbuf = pers.tile([128, 256], BF16)
            bias_sb = pers.tile([128, 2], F32)
            ident = pers.tile([128, 128], F32)
            scr11 = pers.tile([1, 1], F32)

            nc.sync.dma_start(out=wbuf[:], in_=wbuf_d[:, :])
            nc.sync.dma_start(out=ident[:], in_=ident_d[:, :])
            nc.sync.dma_start(out=bias_sb[:, :], in_=bias[:, :])
            nc.sync.dma_start(
                out=value_sb[:], in_=val.rearrange("(c p) d -> p c d", p=128)
            )

            # ---- staged inputs for the projections ----
            WqT_sb = stage.tile([128, DQ // 128, A], F32)
            WkT_sb = stage.tile([128, DK // 128, A], F32)
            qT_sb = stage.tile([128, DQ // 128, TQ], F32)
            kT_sb = stage.tile([128, DK // 128, TK], F32)
            nc.sync.dma_start(out=WqT_sb[:], in_=WqT.rearrange("(c p) a -> p c a", p=128))
            nc.sync.dma_start(out=WkT_sb[:], in_=WkT.rearrange("(c p) a -> p c a", p=128))
            nc.sync.dma_start(out=qT_sb[:], in_=qT.rearrange("(c p) t -> p c t", p=128))
            nc.sync.dma_start(out=kT_sb[:], in_=kT.rearrange("(c p) t -> p c t", p=128))

            # ---- PE observers: absorb one foreign semaphore each so that no
            # later matmul needs more than one sync wait ----
            obs = pobs_pool.tile([1, 1], F32)
            for src in (
                WqT_sb[:, 0, 0:1],
                WkT_sb[:, 0, 0:1],
                qT_sb[:, 0, 0:1],
                kT_sb[:, 0, 0:1],
                value_sb[:, 0, 0:1],
                ident[:, 0:1],
            ):
                nc.tensor.matmul(obs[:], src, src, start=True, stop=True)
            nc.tensor.matmul(
                obs[:], wbuf[:, 0:1], wbuf[:, 0:1], start=True, stop=True
            )
            # ACT observer for the bias DMA (k_pb's activation and the
            # Tanh/Exp zbias reads below must only wait on their data dep)
            nc.scalar.copy(scr11[:], bias_sb[0:1, 0:1])
            zbias = bias_sb[:, 1:2]

            # ---- projections: q_pT[a, tq], k_pb[a, tk] ----
            q_ps = pproj_pool.tile([128, TQ], F32, tag="proj")
            for c in range(DQ // 128):
                nc.tensor.matmul(
                    q_ps[:], WqT_sb[:, c, :], qT_sb[:, c, :],
                    start=(c == 0), stop=(c == DQ // 128 - 1),
                )
            nc.scalar.copy(q_pT[:], q_ps[:])

            k_ps = pproj_pool.tile([128, TK], F32, tag="proj")
            for c in range(DK // 128):
                nc.tensor.matmul(
                    k_ps[:], WkT_sb[:, c, :], kT_sb[:, c, :],
                    start=(c == 0), stop=(c == DK // 128 - 1),
                )
            # fold the additive bias into the psum -> sbuf copy
            nc.scalar.activation(
                k_pb[:], k_ps[:], AF.Identity, bias=bias_sb[:, 0:1], scale=1.0
            )

            # ---- main loop ----
            sums_alloc = []   # allocation-order list of add-instruction lists
            hid_hist = {}     # bidx -> list of instructions writing hid tile
            mm_hist = {}      # bidx -> list of matmul instructions
            for g in range(n_groups):
                plog = plog_pool.tile([128, TK], F32)
                n_fused = FUSED_BLOCKS[g]
                for blk in range(blocks_per_group):
                    bidx = g * blocks_per_group + blk
                    fused = blk >= blocks_per_group - n_fused
                    hid = hid_pool.tile([128, NB * TK], BF16)
                    writers = []
                    if not fused:
                        sums = sums_pool.tile([128, NB * TK], F32)
                        adds = []
                        for jj in range(NB):
                            q = g * 128 + blk * NB + jj
                            h = nc.vector.tensor_scalar_add(
                                sums[:, jj * TK:(jj + 1) * TK],
                                k_pb[:], q_pT[:, q:q + 1],
                            )
                            adds.append(h.ins)
                            # WAW vs the adds that wrote this sums slot
                            # previously: implied through the WAR on the old
                            # tanh's read of the slot.
                            if len(sums_alloc) >= N_SUMS_BUFS:
                                for old in sums_alloc[-N_SUMS_BUFS]:
                                    _demote(h.ins, old)
                        sums_alloc.append(adds)
                        if bidx >= N_HID_BUFS:
                            # fence: carry the hid-slot recycle wait (PE) on
                            # an otherwise wait-free add
                            _add_dep_helper(
                                adds[1], mm_hist[bidx - N_HID_BUFS][-1],
                                sync=True, reason="hid slot recycle fence",
                            )
                        th = nc.scalar.activation(
                            hid[:], sums[:], AF.Tanh, bias=zbias
                        )
                        writers.append(th.ins)
                        if bidx >= N_HID_BUFS:
                            old = bidx - N_HID_BUFS
                            for wi in hid_hist[old]:
                                _demote(th.ins, wi)
                            for mi in mm_hist[old]:
                                _demote(th.ins, mi)
                    else:
                        # fused: tanh(k_pb + q_col) directly, one activation
                        # per query; the first one carries the hid-slot
                        # recycle (PE) wait, later ones have it observed.
                        for jj in range(NB):
                            q = g * 128 + blk * NB + jj
                            th = nc.scalar.activation(
                                hid[:, jj * TK:(jj + 1) * TK], k_pb[:],
                                AF.Tanh, bias=q_pT[:, q:q + 1],
                            )
                            if bidx >= N_HID_BUFS:
                                for wi in hid_hist[bidx - N_HID_BUFS]:
                                    _demote(th.ins, wi)
                                if jj > 0:
                                    for mi in mm_hist[bidx - N_HID_BUFS]:
                                        _demote(th.ins, mi)
                            writers.append(th.ins)
                    hid_hist[bidx] = writers
                    mms = []
                    for jj in range(NB):
                        col = blk * NB + jj
                        m = nc.tensor.matmul(
                            plog[:],
                            wbuf[:, 128 - col:256 - col],
                            hid[:, jj * TK:(jj + 1) * TK],
                            start=(col == 0), stop=(col == 127),
                        )
                        mms.append(m.ins)
                    mm_hist[bidx] = mms

                # ---- epilogue for this 128-query group ----
                expt = epi_pool.tile([128, TK], F32)
                rowsum = small_pool.tile([128, 1], F32)
                nc.scalar.activation(
                    expt[:], plog[:], AF.Exp, bias=zbias, accum_out=rowsum[:, :]
                )
                recip = small_pool.tile([128, 1], F32)
                nc.vector.reciprocal(recip[:, :], rowsum[:, :])
                attn_sb = epi_pool.tile([128, TK], F32)
                nc.vector.tensor_scalar_mul(attn_sb[:], expt[:], recip[:, 0:1])
                nc.sync.dma_start(
                    out=attn_out[g * 128:(g + 1) * 128, :], in_=attn_sb[:]
                )

                out_ps = pout_pool.tile([128, DV], F32)
                for c in range(TK // 128):
                    tp = ptr_pool.tile([128, 128], F32)
                    nc.tensor.transpose(
                        tp[:], attn_sb[:, c * 128:(c + 1) * 128], ident[:]
                    )
                    attnT = attnT_pool.tile([128, 128], F32)
                    nc.vector.tensor_copy(attnT[:], tp[:])
                    nc.tensor.matmul(
                        out_ps[:], attnT[:], value_sb[:, c, :],
                        start=(c == 0), stop=(c == TK // 128 - 1),
                    )
                out_sb = epi_pool.tile([128, DV], F32)
                nc.vector.tensor_copy(out_sb[:], out_ps[:])
                nc.sync.dma_start(
                    out=out_out[g * 128:(g + 1) * 128, :], in_=out_sb[:]
                )

    nc.compile()
    return nc


def _get_nc():
    if "nc" not in _CACHE:
        _CACHE["nc"] = build_nc()
    return _CACHE["nc"]


def make_in_maps(query, key, value, Wq, Wk, bias, w_w, **_):
    WqT = np.ascontiguousarray(Wq.T, dtype=np.float32)
    WkT = np.ascontiguousarray(Wk.T, dtype=np.float32)
    bias_c = np.zeros((A, 2), dtype=np.float32)
    bias_c[:, 0] = np.asarray(bias, dtype=np.float32).reshape(A)
    wbuf_np = np.zeros((128, 256), dtype=np.float32)
    wbuf_np[:, 128] = np.asarray(w_w, dtype=np.float32).reshape(A)
    wbuf_np = wbuf_np.astype(ml_dtypes.bfloat16)
    ident_np = np.eye(128, dtype=np.float32)
    in_maps = []
    for b in range(B):
        in_maps.append({
            "qT": np.ascontiguousarray(query[b].T, dtype=np.float32),
            "kT": np.ascontiguousarray(key[b].T, dtype=np.float32),
            "value": np.ascontiguousarray(value[b], dtype=np.float32),
            "WqT": WqT,
            "WkT": WkT,
            "bias": bias_c,
            "wbuf": wbuf_np,
            "ident": ident_np,
        })
    return in_maps


def run(inputs, trace=False, **kwargs):
    nc = _get_nc()
    in_maps = make_in_maps(**{k: np.asarray(v) for k, v in inputs.items()})
    res = run_bass_kernel_spmd(
        nc, in_maps, list(range(N_CORES)), trace=trace, **kwargs
    )
    output = np.stack([res.results[b]["out"] for b in range(B)])
    attn = np.stack([res.results[b]["attn"] for b in range(B)])
    return (output, attn), res


def kernel(**inputs):
    (output, attn), _ = run(inputs)
    return output, attn
